# revision 25
# baseline (speedup 1.0000x reference)
"""DisentangledProductQuantizer Trainium2 kernel.

Full-input contract: kernel(**inputs) takes the complete tensors and returns
(qf [8,4096,1024] f32, indices [8,4096,4] int32, loss f32 scalar) matching
reference.py semantics.

Sharding: data-parallel over batch. B=8 batches of 4096 tokens -> one batch
per NeuronCore (8 cores). Weights/codebooks replicated. The scalar commitment
loss is reduced on host from per-core partial sums.

Per-core dataflow (N=4096 tokens, G=4 groups, gd=K=256, E=1024):
  T: DMA x rows, PE-transpose (fp32, exact) to feature-major xT.
  A: projT[e,t] = W.T @ xT per group, fp32 matmuls; bias via ACT copy.
  P: per-token |proj|^2 via PE transpose + DVE square-reduce (p2col).
  B: scoresN[t,k] = 2*cross - p2 - c2 (fp32 matmuls + fused DVE epilogue).
     scoresN = -(distances) bit-exactly mirrors the reference rounding.
  M: argmin = DVE max/max_index on scoresN (first-index tie semantics).
  O: one-hot rows from iota==idx, PE-transposed to [K,t].
  D: quantT = C.T @ onehotT in float32r (one-hot exact; codebook values
     rounded ~1e-4, affects only qf).  Straight-through value == quantT.
  E: qf[t,:] = quantT.T @ out_w in float32r (~1.5e-4 rel-to-scale on qf;
     set E_F32R=False for full fp32 at 4x PE cost).
  Commitment loss: sum of min distances = -sum(max(scoresN)) pulled from
     the argmin's max values; host reduces partials in fp64.
  The B matmuls run as a 3-term float32r hi/lo split (13+13 mantissa bits,
     ~1.6e-7 relative) at 1 cyc/row instead of fp32's 4.
"""
import sys
import os

for _p in ("/opt/trn_rl_repo", "/root/.axon_site/_ro/trn_rl_repo"):
    if os.path.isdir(_p) and _p not in sys.path:
        sys.path.insert(0, _p)

import numpy as np
import concourse.bacc as bacc
import concourse.mybir as mybir
from concourse import tile
from concourse import bass_utils
from concourse.alu_op_type import AluOpType

F32 = mybir.dt.float32
F32R = mybir.dt.float32r
U32 = mybir.dt.uint32

B, S, E = 8, 4096, 1024
G, K, GD = 4, 256, 256
N_CORES = 8
N_TOK = (B * S) // N_CORES          # 4096 tokens per core
BETA = 4.0

E_F32R = os.environ.get("E_F32R", "1") == "1"  # output GEMM in float32r

_CACHE = {}


PHASES = ["T", "A", "P", "B", "M", "O", "D", "E"]


def _build(n_tok, e_f32r, outb_zero):
    """Build the per-core Bacc program. Same program runs SPMD on all cores."""
    gate = PHASES.index(os.environ.get("KPHASE", "E"))
    on = lambda ph: PHASES.index(ph) <= gate
    nc = bacc.Bacc("TRN2", target_bir_lowering=False, debug=False,
                   num_devices=N_CORES)
    TILE_T = 256                    # tokens per outer tile
    n_tiles = n_tok // TILE_T
    n_sub = TILE_T // 128           # 4 subchunks of 128 tokens

    d_x = nc.dram_tensor("xin", [n_tok, E], F32, kind="ExternalInput").ap()
    d_pw = nc.dram_tensor("pw", [G, GD, GD], F32, kind="ExternalInput").ap()
    d_pb = nc.dram_tensor("pb", [G, 2, 128, 1], F32, kind="ExternalInput").ap()
    d_cbt2 = nc.dram_tensor("cbt2", [G, 2, 128, K], F32, kind="ExternalInput").ap()
    d_cb = nc.dram_tensor("cb", [G, K, GD], F32, kind="ExternalInput").ap()
    d_c2b = nc.dram_tensor("c2b", [G, 128, K], F32, kind="ExternalInput").ap()
    d_w = nc.dram_tensor("wout", [E, E], F32, kind="ExternalInput").ap()
    d_ob = nc.dram_tensor("outb", [2, 128, 512], F32, kind="ExternalInput").ap()
    d_iota = nc.dram_tensor("iotab", [128, K], F32, kind="ExternalInput").ap()
    d_id = nc.dram_tensor("ident", [128, 128], F32, kind="ExternalInput").ap()

    d_qf = nc.dram_tensor("qf", [n_tok, E], F32, kind="ExternalOutput").ap()
    d_idx = nc.dram_tensor("idx", [n_tok, G], U32, kind="ExternalOutput").ap()
    d_acc = nc.dram_tensor("accv", [128, 1], F32, kind="ExternalOutput").ap()

    EDT = F32R if e_f32r else F32
    Ident = mybir.ActivationFunctionType.Identity
    Square = mybir.ActivationFunctionType.Square

    with tile.TileContext(nc) as tc:
        with tc.tile_pool(name="const", bufs=1) as cp, \
             tc.tile_pool(name="stage", bufs=1) as stg, \
             tc.tile_pool(name="work", bufs=3) as wk, \
             tc.tile_pool(name="big", bufs=2) as bigp, \
             tc.tile_pool(name="psTA", bufs=2, space="PSUM") as psTA, \
             tc.tile_pool(name="psP", bufs=1, space="PSUM") as psP, \
             tc.tile_pool(name="psB", bufs=2, space="PSUM") as psB, \
             tc.tile_pool(name="psOD", bufs=2, space="PSUM") as psOD, \
             tc.tile_pool(name="psE", bufs=1, space="PSUM") as psE:

            # ---- constants -------------------------------------------------
            ident = cp.tile([128, 128], F32, name="ident", tag="ident")
            nc.sync.dma_start(ident[:], d_id)
            iota_b = cp.tile([128, K], F32, name="iota_b", tag="iota_b")
            nc.sync.dma_start(iota_b[:], d_iota)

            identr = cp.tile([128, 128], F32R, name="identr", tag="identr")
            nc.vector.tensor_copy(identr[:], ident[:])
            pw_sb = {}
            for g in range(G):
                for dc in range(2):
                    for eh in range(2):
                        t = cp.tile([128, 128], F32, name=f"pw_{g}_{dc}_{eh}",
                                    tag=f"pw_{g}_{dc}_{eh}")
                        nc.sync.dma_start(
                            t[:], d_pw[g, dc * 128:(dc + 1) * 128,
                                       eh * 128:(eh + 1) * 128])
                        pw_sb[(g, dc, eh)] = t
            pb_sb = {}
            for g in range(G):
                for eh in range(2):
                    t = cp.tile([128, 1], F32, name=f"pb_{g}_{eh}",
                                tag=f"pb_{g}_{eh}")
                    nc.sync.dma_start(t[:], d_pb[g, eh])
                    pb_sb[(g, eh)] = t
            c2h_sb, c2l_sb = {}, {}
            for g in range(G):
                for eh in range(2):
                    s = stg.tile([128, K], F32, name="cbt2_stg",
                                 tag="cbt2_stg")
                    nc.sync.dma_start(s[:], d_cbt2[g, eh])
                    th = cp.tile([128, K], F32R, name=f"c2h_{g}_{eh}",
                                 tag=f"c2h_{g}_{eh}")
                    nc.scalar.activation(th[:], s[:], Ident)
                    tl = cp.tile([128, K], F32R, name=f"c2l_{g}_{eh}",
                                 tag=f"c2l_{g}_{eh}")
                    nc.vector.tensor_tensor(tl[:], s[:], th[:].bitcast(F32),
                                            AluOpType.subtract)
                    c2h_sb[(g, eh)] = th
                    c2l_sb[(g, eh)] = tl
            c2b_sb = {}
            for g in range(G):
                t = cp.tile([128, K], F32, name=f"c2b_{g}", tag=f"c2b_{g}")
                nc.sync.dma_start(t[:], d_c2b[g])
                c2b_sb[g] = t
            cbd_sb = {}
            for g in range(G):
                for kc in range(2):
                    for eh in range(2):
                        s = stg.tile([128, 128], F32, name="cbd_stg",
                                     tag="cbd_stg")
                        nc.sync.dma_start(
                            s[:], d_cb[g, kc * 128:(kc + 1) * 128,
                                       eh * 128:(eh + 1) * 128])
                        t = cp.tile([128, 128], F32R, name=f"cbd_{g}_{kc}_{eh}",
                                    tag=f"cbd_{g}_{kc}_{eh}")
                        nc.vector.tensor_copy(t[:], s[:])
                        cbd_sb[(g, kc, eh)] = t
            outb_sb = {}
            for eo in range(2):
                t = cp.tile([128, 512], F32, name=f"outb_{eo}", tag=f"outb_{eo}")
                nc.sync.dma_start(t[:], d_ob[eo])
                outb_sb[eo] = t
            # out_w chunks; cast to f32r via DVE if E runs in f32r
            we_sb = {}
            for dc in range(8):
                for eo in range(2):
                    if e_f32r:
                        s = stg.tile([128, 512], F32, name="we_stg", tag="we_stg")
                        nc.sync.dma_start(
                            s[:], d_w[dc * 128:(dc + 1) * 128,
                                      eo * 512:(eo + 1) * 512])
                        t = cp.tile([128, 512], EDT, name=f"we_{dc}_{eo}",
                                    tag=f"we_{dc}_{eo}")
                        nc.vector.tensor_copy(t[:], s[:])
                    else:
                        t = cp.tile([128, 512], F32, name=f"we_{dc}_{eo}",
                                    tag=f"we_{dc}_{eo}")
                        nc.sync.dma_start(
                            t[:], d_w[dc * 128:(dc + 1) * 128,
                                      eo * 512:(eo + 1) * 512])
                    we_sb[(dc, eo)] = t

            acc_all = cp.tile([128, 256], F32, name="acc_all", tag="acc_all")
            nc.vector.memset(acc_all[:], 0.0)

            def emit_E(t0, qsteT):
                for j in range(n_sub if on("E") else 0):
                    js = slice(j * 128, (j + 1) * 128)
                    for eo in range(2):
                        pse = psE.tile([128, 512], F32, name="ps_e",
                                       tag="psE")
                        for dc in range(8):
                            nc.tensor.matmul(
                                pse[:], qsteT[dc][:, js], we_sb[(dc, eo)][:],
                                start=(dc == 0), stop=(dc == 7))
                        qfs = wk.tile([128, 512], F32, name="qfs", tag="qfs",
                                      bufs=2)
                        if outb_zero:
                            nc.scalar.copy(qfs[:], pse[:])
                        else:
                            nc.vector.tensor_tensor(qfs[:], pse[:],
                                                    outb_sb[eo][:],
                                                    AluOpType.add)
                        nc.scalar.dma_start(
                            d_qf[t0 + j * 128: t0 + (j + 1) * 128,
                                 eo * 512:(eo + 1) * 512], qfs[:])

            pending_E = []
            cnt = 0
            for it in range(n_tiles):
                t0 = it * TILE_T
                # ---- T: load x and transpose to feature-major --------------
                x_sub = []
                for j in range(n_sub):
                    xt = bigp.tile([128, E], F32, name="x_sub", tag="x_sub",
                                   bufs=2)
                    nc.scalar.dma_start(
                        xt[:], d_x[t0 + j * 128: t0 + (j + 1) * 128, :])
                    x_sub.append(xt)
                xT = [bigp.tile([128, TILE_T], F32, name=f"xT_{dc}",
                                tag=f"xT_{dc}") for dc in range(8)]
                for q in range(8 * n_sub):
                    dc, j = divmod(q, n_sub)
                    pt = psTA.tile([128, 128], F32, name="ps_tr", tag="psTA")
                    nc.tensor.transpose(
                        pt[:], x_sub[j][:, dc * 128:(dc + 1) * 128],
                        ident[:])
                    nc.vector.tensor_copy(
                        xT[dc][:, j * 128:(j + 1) * 128], pt[:])

                # ---- A: projT = W.T @ xT (+bias) ---------------------------
                if not on("A"):
                    continue
                projT = {}
                pjh, pjl = {}, {}
                for gp in range(8):
                    g, eh = divmod(gp, 2)
                    pp = psTA.tile([128, TILE_T], F32, name="ps_pA",
                                   tag="psTA")
                    if True:
                        pv = pp[:]
                        for dcl in range(2):
                            nc.tensor.matmul(
                                pv, pw_sb[(g, dcl, eh)][:],
                                xT[2 * g + dcl][:],
                                start=(dcl == 0), stop=(dcl == 1))
                        pj = bigp.tile([128, TILE_T], F32, name=f"pj_{g}_{eh}",
                                       tag=f"pj_{g}_{eh}")
                        nc.scalar.activation(pj[:], pv, Ident,
                                             bias=pb_sb[(g, eh)][:])
                        ph = bigp.tile([128, TILE_T], F32R,
                                       name=f"ph_{g}_{eh}", tag=f"ph_{g}_{eh}")
                        nc.gpsimd.tensor_copy(ph[:], pj[:])
                        pl = bigp.tile([128, TILE_T], F32R,
                                       name=f"pl_{g}_{eh}", tag=f"pl_{g}_{eh}")
                        nc.gpsimd.tensor_tensor(pl[:], pj[:],
                                                ph[:].bitcast(F32),
                                                AluOpType.subtract)
                        projT[(g, eh)] = pj
                        pjh[(g, eh)] = ph
                        pjl[(g, eh)] = pl

                qsteT = []
                for dc in range(8):
                    q = bigp.tile([128, TILE_T], EDT, name=f"qs_{dc}",
                                  tag=f"qs_{dc}")
                    qsteT.append(q)

                idx_tiles = []
                for j in range(n_sub):
                    ia = wk.tile([128, G], U32, name=f"idx_all_{j}",
                                 tag=f"idx_all_{j}")
                    idx_tiles.append(ia)
                for g in range(G):
                    if not on("P"):
                        break
                    ohT = None
                    pso = None
                    for j in range(n_sub):
                        js = slice(j * 128, (j + 1) * 128)
                        # ---- P: p2col via transpose + square-accum ---------
                        p2col = wk.tile([128, 1], F32, name="p2col",
                                        tag="p2col")
                        p2a = wk.tile([128, 2], F32, name="p2a", tag="p2a")
                        sqs = wk.tile([128, 128], F32, name="sqs", tag="sqs")
                        ptk = psP.tile([128, 256], F32, name="ps_ptk",
                                       tag="psP")
                        for eh in range(2):
                            pv = ptk[:, eh * 128:(eh + 1) * 128]
                            nc.tensor.transpose(
                                pv, projT[(g, eh)][:, js], ident[:])
                            nc.scalar.activation(
                                sqs[:], pv, Square,
                                accum_out=p2a[:, eh:eh + 1])
                        nc.vector.tensor_tensor(
                            p2col[:], p2a[:, 0:1], p2a[:, 1:2], AluOpType.add)
                        if not on("B"):
                            continue
                        # ---- B: scoresN = 2*cross - p2 - c2 ----------------
                        psc = psB.tile([128, K], F32, name="ps_sc",
                                       tag="psB")
                        for eh in range(2):
                            nc.tensor.matmul(
                                psc[:], pjh[(g, eh)][:, js],
                                c2h_sb[(g, eh)][:],
                                start=(eh == 0), stop=False)
                            nc.tensor.matmul(
                                psc[:], pjh[(g, eh)][:, js],
                                c2l_sb[(g, eh)][:], start=False, stop=False)
                            nc.tensor.matmul(
                                psc[:], pjl[(g, eh)][:, js],
                                c2h_sb[(g, eh)][:],
                                start=False, stop=(eh == 1))
                        nsb = wk.tile([128, K], F32, name="nsb", tag="nsb")
                        nc.vector.scalar_tensor_tensor(
                            nsb[:], psc[:], p2col[:], c2b_sb[g][:],
                            AluOpType.subtract, AluOpType.subtract)
                        if not on("M"):
                            continue
                        # ---- M: argmin (max of negated scores) -------------
                        m8 = wk.tile([128, 8], F32, name="m8", tag="m8")
                        i8 = wk.tile([128, 8], U32, name="i8", tag="i8")
                        nc.vector.max(m8[:], nsb[:])
                        nc.vector.max_index(i8[:], m8[:], nsb[:])
                        nc.gpsimd.tensor_copy(acc_all[:, cnt:cnt + 1],
                                              m8[:, 0:1])
                        cnt += 1
                        nc.gpsimd.tensor_copy(idx_tiles[j][:, g:g + 1],
                                              i8[:, 0:1])
                        idxf = wk.tile([128, 1], F32, name="idxf", tag="idxf")
                        nc.gpsimd.tensor_copy(idxf[:], i8[:, 0:1])
                        if not on("O"):
                            continue
                        # ---- O: one-hot rows, transposed (batch 2 j) -------
                        oh = wk.tile([128, K], F32R, name="oh", tag="oh")
                        nc.gpsimd.tensor_scalar(oh[:], iota_b[:], idxf[:],
                                                None, AluOpType.is_equal)
                        if ohT is None:
                            ohT = [wk.tile([128, TILE_T], F32R,
                                           name=f"ohT_{kc}", tag=f"ohT_{kc}",
                                           bufs=2)
                                   for kc in range(2)]
                            pod = psOD.tile([128, 512], F32, name="ps_od",
                                            tag="psOD")
                            pso = pod
                        for kc in range(2):
                            pv = pso[:, (kc * n_sub + j) * 128:
                                     (kc * n_sub + j + 1) * 128]
                            nc.tensor.transpose(
                                pv.bitcast(F32R),
                                oh[:, kc * 128:(kc + 1) * 128],
                                identr[:])
                            nc.vector.tensor_copy(ohT[kc][:, js],
                                                  pv.bitcast(F32R))
                    if not on("D") or ohT is None:
                        continue
                    # ---- D: quantT (f32r, Nf=256); STE == quant ------------
                    podq = psOD.tile([128, 512], F32, name="ps_odq",
                                     tag="psOD")
                    for eh in range(2):
                        pv = podq[:, eh * 256:(eh + 1) * 256]
                        for kc in range(2):
                            nc.tensor.matmul(
                                pv, cbd_sb[(g, kc, eh)][:], ohT[kc][:],
                                start=(kc == 0), stop=(kc == 1))
                        nc.vector.tensor_copy(qsteT[2 * g + eh][:], pv)
                if on("M"):
                    for j in range(n_sub):
                        nc.scalar.dma_start(
                            d_idx[t0 + j * 128: t0 + (j + 1) * 128, :],
                            idx_tiles[j][:])

                # ---- E emission deferred by one tile (see emit_E) ----------
                pending_E.append((t0, qsteT))
                if len(pending_E) > 1:
                    emit_E(*pending_E.pop(0))

            for args in pending_E:
                emit_E(*args)
            pending_E = []

            accv = cp.tile([128, 1], F32, name="accv", tag="accv")
            nc.vector.tensor_reduce(accv[:], acc_all[:], mybir.AxisListType.X,
                                    AluOpType.add)
            nc.sync.dma_start(d_acc, accv[:])

    nc.compile()
    return nc


def _c2_fp32(codebooks):
    """c2 = sum(codebooks**2, -1) replicating the reference's fp32 path."""
    try:
        import jax
        import jax.numpy as jnp
        cpu = jax.devices("cpu")[0]
        with jax.default_device(cpu):
            cbj = jax.device_put(np.asarray(codebooks, np.float32), cpu)
            return np.asarray(jnp.sum(cbj * cbj, axis=-1))
    except Exception:
        cb = np.asarray(codebooks, np.float32)
        return np.add.reduce(cb * cb, axis=-1, dtype=np.float32)


def _prep_shared(proj_w, proj_b, codebooks, out_w, out_b):
    pw = np.ascontiguousarray(proj_w, np.float32)
    pb = np.ascontiguousarray(proj_b, np.float32).reshape(G, 2, 128, 1)
    cb = np.ascontiguousarray(codebooks, np.float32)
    cbt2 = np.ascontiguousarray(
        (2.0 * cb).transpose(0, 2, 1).reshape(G, 2, 128, K))
    c2 = _c2_fp32(cb)                                        # [G, K]
    c2b = np.ascontiguousarray(
        np.broadcast_to(c2[:, None, :], (G, 128, K)), np.float32)
    w = np.ascontiguousarray(out_w, np.float32)
    ob = np.ascontiguousarray(
        np.broadcast_to(np.asarray(out_b, np.float32).reshape(2, 1, 512),
                        (2, 128, 512)))
    iota = np.ascontiguousarray(
        np.broadcast_to(np.arange(K, dtype=np.float32)[None, :], (128, K)))
    ident = np.eye(128, dtype=np.float32)
    return dict(pw=pw, pb=pb, cbt2=cbt2, cb=cb, c2b=c2b, wout=w, outb=ob,
                iotab=iota, ident=ident)


def run(features, proj_w, proj_b, codebooks, out_w, out_b,
        n_tok=N_TOK, e_f32r=E_F32R, trace=False):
    outb_zero = not np.any(np.asarray(out_b))
    key = (n_tok, e_f32r, outb_zero)
    if key not in _CACHE:
        _CACHE[key] = _build(n_tok, e_f32r, outb_zero)
    nc = _CACHE[key]

    shared = _prep_shared(proj_w, proj_b, codebooks, out_w, out_b)
    feats = np.ascontiguousarray(features, np.float32).reshape(-1, E)
    n_cores = N_CORES
    in_maps = []
    for c in range(n_cores):
        m = dict(shared)
        m["xin"] = np.ascontiguousarray(feats[c * n_tok:(c + 1) * n_tok])
        in_maps.append(m)

    res = bass_utils.run_bass_kernel_spmd(
        nc, in_maps, core_ids=list(range(n_cores)), trace=trace)

    qf = np.concatenate([res.results[c]["qf"] for c in range(n_cores)], axis=0)
    idx = np.concatenate([res.results[c]["idx"] for c in range(n_cores)],
                         axis=0).astype(np.int32)
    total = np.float64(0.0)
    for c in range(n_cores):
        total -= res.results[c]["accv"].astype(np.float64).sum()
    n_el = np.float64(n_cores * n_tok * E)
    loss = np.float32(BETA * np.float32(total / n_el))
    return qf, idx, loss, res


def timed_run(features, proj_w, proj_b, codebooks, out_w, out_b,
              n_tok=N_TOK, e_f32r=E_F32R, reps=6):
    """Time warm SPMD executions (inputs resident on device, donated outs)."""
    import time as _time
    import jax
    from jax.sharding import Mesh, PartitionSpec
    from jax.experimental.shard_map import shard_map
    from concourse import bass2jax
    from concourse.bass2jax import _bass_exec_p

    outb_zero = not np.any(np.asarray(out_b))
    key = (n_tok, e_f32r, outb_zero)
    if key not in _CACHE:
        _CACHE[key] = _build(n_tok, e_f32r, outb_zero)
    nc = _CACHE[key]
    bass2jax.install_neuronx_cc_hook()

    shared = _prep_shared(proj_w, proj_b, codebooks, out_w, out_b)
    feats = np.ascontiguousarray(features, np.float32).reshape(-1, E)
    in_maps = []
    for c in range(N_CORES):
        m = dict(shared)
        m["xin"] = np.ascontiguousarray(feats[c * n_tok:(c + 1) * n_tok])
        in_maps.append(m)

    import concourse.mybir as _mb
    pname = nc.partition_id_tensor.name if nc.partition_id_tensor else None
    in_names, out_names, out_avals = [], [], []
    for alloc in nc.m.functions[0].allocations:
        if not isinstance(alloc, _mb.MemoryLocationSet):
            continue
        name = alloc.memorylocations[0].name
        if alloc.kind == "ExternalInput":
            if name != pname:
                in_names.append(name)
        elif alloc.kind == "ExternalOutput":
            out_names.append(name)
            out_avals.append(
                jax.core.ShapedArray(tuple(alloc.tensor_shape),
                                     _mb.dt.np(alloc.dtype)))
    n_params = len(in_names)
    n_outs = len(out_avals)
    all_names = in_names + out_names
    if pname is not None:
        all_names = all_names + [pname]

    def _body(*args):
        operands = list(args)
        if pname is not None:
            operands.append(bass2jax.partition_id_tensor())
        outs = _bass_exec_p.bind(
            *operands, out_avals=tuple(out_avals), in_names=tuple(all_names),
            out_names=tuple(out_names), lowering_input_output_aliases=(),
            sim_require_finite=True, sim_require_nnan=True, nc=nc)
        return tuple(outs)

    devices = jax.devices()[:N_CORES]
    mesh = Mesh(np.asarray(devices), ("core",))
    in_specs = (PartitionSpec("core"),) * (n_params + n_outs)
    out_specs = (PartitionSpec("core"),) * n_outs
    fn = jax.jit(shard_map(_body, mesh=mesh, in_specs=in_specs,
                           out_specs=out_specs, check_rep=False),
                 donate_argnums=tuple(range(n_params, n_params + n_outs)),
                 keep_unused=True)
    sharding = jax.sharding.NamedSharding(mesh, PartitionSpec("core"))
    concat_in = [
        jax.device_put(
            np.concatenate([np.asarray(in_maps[c][nm]) for c in range(N_CORES)],
                           axis=0), sharding)
        for nm in in_names]
    times = []
    for _ in range(reps):
        zeros = [jax.device_put(
            np.zeros((N_CORES * a.shape[0], *a.shape[1:]), a.dtype), sharding)
            for a in out_avals]
        jax.block_until_ready(zeros)
        t0 = _time.perf_counter()
        outs = fn(*concat_in, *zeros)
        jax.block_until_ready(outs)
        times.append(_time.perf_counter() - t0)
    return times


def kernel(features, proj_w, proj_b, codebooks, out_w, out_b):
    Bf, Sf, Ef = features.shape
    qf, idx, loss, _ = run(features, proj_w, proj_b, codebooks, out_w, out_b)
    qf = qf.reshape(Bf, Sf, Ef)
    idx = idx.reshape(Bf, Sf, G)
    return qf, idx, loss
